# revision 1
# baseline (speedup 1.0000x reference)
"""Trainium2 Bass kernel for batched Clifford (Cl(3,1)) geometric product.

out[n, c] = sum_{i,j} CAYLEY[i, j, c] * a[n, i] * b[n, j]

Strategy: Cl(3,1) is isomorphic to M4(R) (real 4x4 matrices). Via a fixed
linear basis change Phi (signed, sparse), the 256-term bilinear blade
product becomes a per-token 4x4 matrix product (64 multiplies). All linear
maps (Phi on both inputs, the k-contraction fused with Phi^-1) run on the
TensorEngine against constant matrices; the only elementwise work is one
[128,512] multiply per 1024 tokens on the VectorEngine.

Data parallel over 8 NeuronCores: each core handles 131072 rows.
"""
import sys

sys.path.insert(0, "/opt/trn_rl_repo")

import numpy as np

N_TOTAL = 1048576
N_CORES = 8
ROWS_PER_CORE = N_TOTAL // N_CORES   # 131072
P = 128
F = 512
NT = ROWS_PER_CORE // 4096           # 32 big tiles of 4096 tokens


# ---------------------------------------------------------------------------
# Constant construction: gamma matrices, Phi iso, expansion/contraction mats
# ---------------------------------------------------------------------------
def _build_consts():
    X = np.array([[0.0, 1.0], [1.0, 0.0]])
    Z = np.array([[1.0, 0.0], [0.0, -1.0]])
    E = np.array([[0.0, 1.0], [-1.0, 0.0]])
    I2 = np.eye(2)
    # generators of Cl(3,1): squares +1,+1,+1,-1, pairwise anticommuting
    g = [np.kron(X, I2), np.kron(Z, I2), np.kron(E, E), np.kron(E, X)]
    M = []
    for I in range(16):
        m = np.eye(4)
        for bit in range(4):
            if (I >> bit) & 1:
                m = m @ g[bit]
        M.append(m)
    Phi = np.stack([m.reshape(16) for m in M], axis=1)   # [(r,c), blade]
    PhiInv = Phi.T / 4.0                                 # orthogonal basis

    Ea = np.zeros((32, 128), np.float32)
    Eb = np.zeros((32, 128), np.float32)
    K4 = np.zeros((128, 32), np.float32)
    for v in range(2):
        for r in range(4):
            for k in range(4):
                for c in range(4):
                    col = v * 64 + r * 16 + k * 4 + c
                    for f in range(16):
                        Ea[v * 16 + f, col] = Phi[r * 4 + k, f]
                        Eb[v * 16 + f, col] = Phi[k * 4 + c, f]
                    for cb in range(16):
                        K4[col, v * 16 + cb] = PhiInv[cb, r * 4 + c]
    Ea4 = np.concatenate([Ea] * 4, axis=0).astype(np.float32)
    Eb4 = np.concatenate([Eb] * 4, axis=0).astype(np.float32)
    return Ea4, Eb4, K4.astype(np.float32), np.eye(128, dtype=np.float32)


def build_program(rows_per_core=ROWS_PER_CORE, repeats=1, dyn_repeats=None,
                  ablate=0, bf16=False, back_first=False, paired=False, cast_load=True,
                  sb_bufs=4):
    import contextlib

    import concourse.bacc as bacc
    import concourse.mybir as mybir
    from concourse.tile import TileContext

    nt = rows_per_core // 4096
    nc = bacc.Bacc("TRN2", target_bir_lowering=False)
    dt = mybir.dt.float32
    dm = mybir.dt.bfloat16 if bf16 else mybir.dt.float32
    a = nc.dram_tensor("a", [rows_per_core, 16], dt, kind="ExternalInput")
    b = nc.dram_tensor("b", [rows_per_core, 16], dt, kind="ExternalInput")
    cEa = nc.dram_tensor("cEa", [128, 128], dm, kind="ExternalInput")
    cEb = nc.dram_tensor("cEb", [128, 128], dm, kind="ExternalInput")
    cK4 = nc.dram_tensor("cK4", [128, 32], dm, kind="ExternalInput")
    cI = nc.dram_tensor("cI", [128, 128], dm, kind="ExternalInput")
    cI32 = nc.dram_tensor("cI32", [128, 128], dt, kind="ExternalInput")
    o = nc.dram_tensor("o", [rows_per_core, 16], dt, kind="ExternalOutput")

    af = a.rearrange("(n g w) c -> n g (w c)", g=P, w=32)
    bf = b.rearrange("(n g w) c -> n g (w c)", g=P, w=32)
    of = o.rearrange("(n g w) c -> n g (w c)", g=P, w=32)

    with TileContext(nc) as tc:
        with tc.tile_pool(name="const", bufs=1) as cpool, \
             tc.tile_pool(name="sb", bufs=sb_bufs) as sb, \
             tc.tile_pool(name="ps1", bufs=2, space="PSUM") as ps1, \
             tc.tile_pool(name="ps2", bufs=2, space="PSUM") as ps2:
            tEa = cpool.tile([128, 128], dm)
            tEb = cpool.tile([128, 128], dm)
            tK4 = cpool.tile([128, 32], dm)
            tI = cpool.tile([128, 128], dm)
            tI32 = cpool.tile([128, 128], dt)
            nc.sync.dma_start(tEa[:, :], cEa[:, :])
            nc.sync.dma_start(tEb[:, :], cEb[:, :])
            nc.sync.dma_start(tK4[:, :], cK4[:, :])
            nc.sync.dma_start(tI[:, :], cI[:, :])
            nc.sync.dma_start(tI32[:, :], cI32[:, :])

            def emit_front(n):
                """loads, transposes, expansions, muls, K4 contraction.
                Returns the pout2 psum tile holding the tile's result."""
                dl = dm if (bf16 and cast_load) else dt
                ta = sb.tile([P, F], dl, tag="ta", name="ta")
                tb = sb.tile([P, F], dl, tag="tb", name="tb")
                dma_in = nc.gpsimd if (bf16 and cast_load) else nc.sync
                dma_in.dma_start(ta[:, :], af[n])
                dma_in.dma_start(tb[:, :], bf[n])
                if ablate >= 6:
                    nc.sync.dma_start(of[n], ta[:, :])
                    return None

                if bf16 and paired:
                    # paired middle: all 8 transposed chunks live in ONE psum
                    # bank ([128,1024] bf16); expansions run in j-pairs into
                    # 2-bank psum tiles so every evacuation/mul is 1024 wide.
                    pabT = ps1.tile([P, 2 * F], dm, tag="pT", name="pabT")
                    for c in range(4):
                        nc.tensor.transpose(pabT[:, 128 * c:128 * (c + 1)],
                                            ta[:, 128 * c:128 * (c + 1)], tI[:, :])
                        nc.tensor.transpose(pabT[:, 512 + 128 * c:512 + 128 * (c + 1)],
                                            tb[:, 128 * c:128 * (c + 1)], tI[:, :])
                    sabT = sb.tile([P, 2 * F], dm, tag="sabT", name="sabT")
                    nc.vector.tensor_copy(sabT[:, :], pabT[:, :])
                    if ablate >= 5:
                        nc.sync.dma_start(of[n], sabT[:, 0:F])
                        return None
                    spps = []
                    for half in range(2):
                        pA2 = ps2.tile([P, 2 * F], dt, tag="pA", name="pA2", bufs=1)
                        pB2 = ps2.tile([P, 2 * F], dt, tag="pB", name="pB2", bufs=1)
                        for jj in range(2):
                            j = 2 * half + jj
                            js = slice(32 * j, 32 * (j + 1))
                            nc.tensor.matmul(pA2[:, F * jj:F * (jj + 1)],
                                             tEa[js, :], sabT[js, 0:F],
                                             start=True, stop=True,
                                             tile_position=(32 * j, 0))
                            nc.tensor.matmul(pB2[:, F * jj:F * (jj + 1)],
                                             tEb[js, :], sabT[js, F:2 * F],
                                             start=True, stop=True,
                                             tile_position=(32 * j, 0))
                        sA2 = sb.tile([P, 2 * F], dm, tag="sA", name="sA2")
                        nc.scalar.copy(sA2[:, :], pA2[:, :])
                        spp = sb.tile([P, 2 * F], dm, tag="spp", name="spp",
                                      bufs=3)
                        nc.vector.tensor_mul(spp[:, :], sA2[:, :], pB2[:, :])
                        spps.append(spp)
                    pout2 = ps1.tile([P, F], dt, tag="pout2", name="pout2")
                    for j in range(4):
                        nc.tensor.matmul(pout2[32 * j:32 * (j + 1), :], tK4[:, :],
                                         spps[j // 2][:, F * (j % 2):F * (j % 2 + 1)],
                                         start=True, stop=True,
                                         tile_position=(0, 32 * j))
                    return pout2

                tIu = tI if (bf16 and cast_load) else tI32
                saT = sb.tile([P, F], dm, tag="saT", name="saT")
                sbT = sb.tile([P, F], dm, tag="sbT", name="sbT")
                paT = ps1.tile([P, F], dl, tag="pT", name="paT")
                for c in range(4):
                    nc.tensor.transpose(paT[:, 128 * c:128 * (c + 1)],
                                        ta[:, 128 * c:128 * (c + 1)], tIu[:, :])
                nc.scalar.copy(saT[:, :], paT[:, :])
                pbT = ps1.tile([P, F], dl, tag="pT", name="pbT")
                for c in range(4):
                    nc.tensor.transpose(pbT[:, 128 * c:128 * (c + 1)],
                                        tb[:, 128 * c:128 * (c + 1)], tIu[:, :])
                nc.vector.tensor_copy(sbT[:, :], pbT[:, :])
                if ablate >= 5:
                    nc.sync.dma_start(of[n], saT[:, :])
                    return None

                pAs, pBs = [None] * 4, [None] * 4

                def emit_exp(j):
                    pAs[j] = ps2.tile([P, F], dt, tag="pA", name="pA")
                    pBs[j] = ps2.tile([P, F], dt, tag="pB", name="pB")
                    nc.tensor.matmul(pAs[j][:, :], tEa[32 * j:32 * (j + 1), :],
                                     saT[32 * j:32 * (j + 1), :],
                                     start=True, stop=True,
                                     tile_position=(32 * j, 0))
                    nc.tensor.matmul(pBs[j][:, :], tEb[32 * j:32 * (j + 1), :],
                                     sbT[32 * j:32 * (j + 1), :],
                                     start=True, stop=True,
                                     tile_position=(32 * j, 0))

                emit_exp(0)
                emit_exp(1)
                spps = [None] * 4
                for j in range(4):
                    sA = sb.tile([P, F], dm, tag="sA", name="sA")
                    nc.scalar.copy(sA[:, :], pAs[j][:, :])
                    spps[j] = sb.tile([P, F], dm, tag="spp", name="spp", bufs=5)
                    nc.vector.tensor_mul(spps[j][:, :], sA[:, :], pBs[j][:, :])
                    if j + 2 < 4:
                        emit_exp(j + 2)
                pout2 = ps1.tile([P, F], dt, tag="pout2", name="pout2")
                for j in range(4):
                    nc.tensor.matmul(pout2[32 * j:32 * (j + 1), :], tK4[:, :],
                                     spps[j][:, :], start=True, stop=True,
                                     tile_position=(0, 32 * j))
                return pout2

            def emit_back(n, pout2):
                sout2 = sb.tile([P, F], dt, tag="sout2", name="sout2")
                nc.scalar.copy(sout2[:, :], pout2[:, :])
                if ablate >= 2:
                    nc.sync.dma_start(of[n], sout2[:, :])
                    return
                poTs = ps1.tile([P, F], dt, tag="pout2", name="poTs")
                for c in range(4):
                    nc.tensor.transpose(poTs[:, 128 * c:128 * (c + 1)],
                                        sout2[:, 128 * c:128 * (c + 1)], tI32[:, :])
                onat = sb.tile([P, F], dt, tag="onat", name="onat")
                nc.vector.tensor_copy(onat[:, :], poTs[:, :])
                nc.sync.dma_start(of[n], onat[:, :])

            loop_cm = (tc.For_i(0, dyn_repeats, 1) if dyn_repeats
                       else contextlib.nullcontext())
            with loop_cm:
              for _rep in range(repeats):
                prev = None
                for n in range(nt):
                    if back_first and prev is not None:
                        emit_back(prev[0], prev[1])
                        prev = None
                    pout2 = emit_front(n)
                    if prev is not None:
                        emit_back(prev[0], prev[1])
                    prev = (n, pout2) if pout2 is not None else None
                if prev is not None:
                    emit_back(prev[0], prev[1])

    nc.finalize()
    return nc


_CACHE = {}


def make_in_maps(a, b, bf16=False):
    import ml_dtypes
    Ea4, Eb4, K4c, I128 = _build_consts()
    md = ml_dtypes.bfloat16 if bf16 else np.float32
    consts = {"cEa": Ea4.astype(md), "cEb": Eb4.astype(md),
              "cK4": K4c.astype(md), "cI": I128.astype(md), "cI32": I128}
    in_maps = []
    for i in range(N_CORES):
        sl = slice(i * ROWS_PER_CORE, (i + 1) * ROWS_PER_CORE)
        in_maps.append({"a": a[sl], "b": b[sl], **consts})
    return in_maps


USE_BF16 = False


def kernel(a, b):
    from concourse.bass_utils import run_bass_kernel_spmd

    a = np.ascontiguousarray(np.asarray(a, dtype=np.float32))
    b = np.ascontiguousarray(np.asarray(b, dtype=np.float32))
    assert a.shape == (N_TOTAL, 16) and b.shape == (N_TOTAL, 16)
    if "nc" not in _CACHE:
        _CACHE["nc"] = build_program(bf16=USE_BF16)
    nc = _CACHE["nc"]
    in_maps = make_in_maps(a, b, bf16=USE_BF16)
    res = run_bass_kernel_spmd(nc, in_maps, core_ids=list(range(N_CORES)))
    return np.concatenate([res.results[i]["o"] for i in range(N_CORES)], axis=0)



# revision 17
# speedup vs baseline: 1091.3152x; 1091.3152x over previous
"""Trainium2 Bass kernel for batched Clifford (Cl(3,1)) geometric product.

out[n, c] = sum_{i,j} CAYLEY[i, j, c] * a[n, i] * b[n, j]

Algorithm: Cl(3,1) is isomorphic to M4(R) (real 4x4 matrices). Via a fixed
linear basis change Phi (signed, sparse), the 256-term bilinear blade
product becomes a per-token 4x4 matrix product (64 multiplies). All linear
maps (Phi on both inputs via the Ea/Eb expansion matrices, and the
k-contraction fused with Phi^-1 via K4) run on the TensorEngine against
constant stationary matrices; the per-token data*data multiply runs on the
VectorEngine. Compute is bf16 (tolerance 2e-2; measured ~4e-3 Frobenius).

Memory system (the limiter for this problem): per-dma_start fixed cost is
~2us completion latency, so HBM traffic moves in 2MB megatile DMAs (8
compute tiles per transfer) with a-loads on the SP HWDGE ring, b-loads on
the ACT HWDGE ring, and stores on a third ring, giving a measured DMA floor
of ~73us/pass/core vs 251us with 256KB-grain transfers.

Data parallel over 8 NeuronCores: each core handles 131072 rows.
"""
import sys

sys.path.insert(0, "/opt/trn_rl_repo")

import numpy as np

N_TOTAL = 1048576
N_CORES = 8
ROWS_PER_CORE = N_TOTAL // N_CORES   # 131072
P = 128
F = 512
NT = ROWS_PER_CORE // 4096           # 32 big tiles of 4096 tokens


# ---------------------------------------------------------------------------
# Constant construction: gamma matrices, Phi iso, expansion/contraction mats
# ---------------------------------------------------------------------------
def _build_consts():
    X = np.array([[0.0, 1.0], [1.0, 0.0]])
    Z = np.array([[1.0, 0.0], [0.0, -1.0]])
    E = np.array([[0.0, 1.0], [-1.0, 0.0]])
    I2 = np.eye(2)
    # generators of Cl(3,1): squares +1,+1,+1,-1, pairwise anticommuting
    g = [np.kron(X, I2), np.kron(Z, I2), np.kron(E, E), np.kron(E, X)]
    M = []
    for I in range(16):
        m = np.eye(4)
        for bit in range(4):
            if (I >> bit) & 1:
                m = m @ g[bit]
        M.append(m)
    Phi = np.stack([m.reshape(16) for m in M], axis=1)   # [(r,c), blade]
    PhiInv = Phi.T / 4.0                                 # orthogonal basis

    Ea = np.zeros((32, 128), np.float32)
    Eb = np.zeros((32, 128), np.float32)
    K4 = np.zeros((128, 32), np.float32)
    for v in range(2):
        for r in range(4):
            for k in range(4):
                for c in range(4):
                    col = v * 64 + r * 16 + k * 4 + c
                    for f in range(16):
                        Ea[v * 16 + f, col] = Phi[r * 4 + k, f]
                        Eb[v * 16 + f, col] = Phi[k * 4 + c, f]
                    for cb in range(16):
                        K4[col, v * 16 + cb] = PhiInv[cb, r * 4 + c]
    Ea4 = np.concatenate([Ea] * 4, axis=0).astype(np.float32)
    Eb4 = np.concatenate([Eb] * 4, axis=0).astype(np.float32)
    return Ea4, Eb4, K4.astype(np.float32), np.eye(128, dtype=np.float32)


def build_program(rows_per_core=ROWS_PER_CORE, repeats=1, dyn_repeats=None,
                  ablate=0, bf16=False, back_first=False, paired=False, cast_load=True,
                  sb_bufs=4, obf=False):
    import contextlib

    import concourse.bacc as bacc
    import concourse.mybir as mybir
    from concourse.tile import TileContext

    nt = rows_per_core // 4096
    nc = bacc.Bacc("TRN2", target_bir_lowering=False)
    dt = mybir.dt.float32
    dm = mybir.dt.bfloat16 if bf16 else mybir.dt.float32
    a = nc.dram_tensor("a", [rows_per_core, 16], dt, kind="ExternalInput")
    b = nc.dram_tensor("b", [rows_per_core, 16], dt, kind="ExternalInput")
    cEa = nc.dram_tensor("cEa", [128, 128], dm, kind="ExternalInput")
    cEb = nc.dram_tensor("cEb", [128, 128], dm, kind="ExternalInput")
    cK4 = nc.dram_tensor("cK4", [128, 32], dm, kind="ExternalInput")
    cI = nc.dram_tensor("cI", [128, 128], dm, kind="ExternalInput")
    cI32 = nc.dram_tensor("cI32", [128, 128], dt, kind="ExternalInput")
    o = nc.dram_tensor("o", [rows_per_core, 16], dt, kind="ExternalOutput")

    af = a.rearrange("(n g w) c -> n g (w c)", g=P, w=32)
    bf = b.rearrange("(n g w) c -> n g (w c)", g=P, w=32)
    of = o.rearrange("(n g w) c -> n g (w c)", g=P, w=32)

    with TileContext(nc) as tc:
        with tc.tile_pool(name="const", bufs=1) as cpool, \
             tc.tile_pool(name="sb", bufs=sb_bufs) as sb, \
             tc.tile_pool(name="ps1", bufs=2, space="PSUM") as ps1, \
             tc.tile_pool(name="ps2", bufs=2, space="PSUM") as ps2:
            tEa = cpool.tile([128, 128], dm)
            tEb = cpool.tile([128, 128], dm)
            tK4 = cpool.tile([128, 32], dm)
            tI = cpool.tile([128, 128], dm)
            tI32 = cpool.tile([128, 128], dt)
            nc.sync.dma_start(tEa[:, :], cEa[:, :])
            nc.sync.dma_start(tEb[:, :], cEb[:, :])
            nc.sync.dma_start(tK4[:, :], cK4[:, :])
            nc.sync.dma_start(tI[:, :], cI[:, :])
            nc.sync.dma_start(tI32[:, :], cI32[:, :])

            def emit_front(n):
                """loads, transposes, expansions, muls, K4 contraction.
                Returns the pout2 psum tile holding the tile's result."""
                dl = dm if (bf16 and cast_load) else dt
                ta = sb.tile([P, F], dl, tag="ta", name="ta")
                tb = sb.tile([P, F], dl, tag="tb", name="tb")
                dma_in = nc.gpsimd if (bf16 and cast_load) else nc.sync
                dma_in.dma_start(ta[:, :], af[n])
                dma_in.dma_start(tb[:, :], bf[n])
                if ablate >= 6:
                    nc.sync.dma_start(of[n], ta[:, :])
                    return None

                if bf16 and paired:
                    # paired middle: all 8 transposed chunks live in ONE psum
                    # bank ([128,1024] bf16); expansions run in j-pairs into
                    # 2-bank psum tiles so every evacuation/mul is 1024 wide.
                    pabT = ps1.tile([P, 2 * F], dm, tag="pT", name="pabT")
                    for c in range(4):
                        nc.tensor.transpose(pabT[:, 128 * c:128 * (c + 1)],
                                            ta[:, 128 * c:128 * (c + 1)], tI[:, :])
                        nc.tensor.transpose(pabT[:, 512 + 128 * c:512 + 128 * (c + 1)],
                                            tb[:, 128 * c:128 * (c + 1)], tI[:, :])
                    sabT = sb.tile([P, 2 * F], dm, tag="sabT", name="sabT")
                    nc.vector.tensor_copy(sabT[:, :], pabT[:, :])
                    if ablate >= 5:
                        nc.sync.dma_start(of[n], sabT[:, 0:F])
                        return None
                    spps = []
                    for half in range(2):
                        pA2 = ps2.tile([P, 2 * F], dt, tag="pA", name="pA2", bufs=1)
                        pB2 = ps2.tile([P, 2 * F], dt, tag="pB", name="pB2", bufs=1)
                        for jj in range(2):
                            j = 2 * half + jj
                            js = slice(32 * j, 32 * (j + 1))
                            nc.tensor.matmul(pA2[:, F * jj:F * (jj + 1)],
                                             tEa[js, :], sabT[js, 0:F],
                                             start=True, stop=True,
                                             tile_position=(32 * j, 0))
                            nc.tensor.matmul(pB2[:, F * jj:F * (jj + 1)],
                                             tEb[js, :], sabT[js, F:2 * F],
                                             start=True, stop=True,
                                             tile_position=(32 * j, 0))
                        sA2 = sb.tile([P, 2 * F], dm, tag="sA", name="sA2")
                        nc.scalar.copy(sA2[:, :], pA2[:, :])
                        spp = sb.tile([P, 2 * F], dm, tag="spp", name="spp",
                                      bufs=3)
                        nc.vector.tensor_mul(spp[:, :], sA2[:, :], pB2[:, :])
                        spps.append(spp)
                    pout2 = ps1.tile([P, F], dt, tag="pout2", name="pout2")
                    for j in range(4):
                        nc.tensor.matmul(pout2[32 * j:32 * (j + 1), :], tK4[:, :],
                                         spps[j // 2][:, F * (j % 2):F * (j % 2 + 1)],
                                         start=True, stop=True,
                                         tile_position=(0, 32 * j))
                    return pout2

                tIu = tI if (bf16 and cast_load) else tI32
                saT = sb.tile([P, F], dm, tag="saT", name="saT")
                sbT = sb.tile([P, F], dm, tag="sbT", name="sbT")
                paT = ps1.tile([P, F], dl, tag="pT", name="paT")
                for c in range(4):
                    nc.tensor.transpose(paT[:, 128 * c:128 * (c + 1)],
                                        ta[:, 128 * c:128 * (c + 1)], tIu[:, :])
                nc.scalar.copy(saT[:, :], paT[:, :])
                pbT = ps1.tile([P, F], dl, tag="pT", name="pbT")
                for c in range(4):
                    nc.tensor.transpose(pbT[:, 128 * c:128 * (c + 1)],
                                        tb[:, 128 * c:128 * (c + 1)], tIu[:, :])
                nc.vector.tensor_copy(sbT[:, :], pbT[:, :])
                if ablate >= 5:
                    nc.sync.dma_start(of[n], saT[:, :])
                    return None

                pAs, pBs = [None] * 4, [None] * 4

                def emit_exp(j):
                    pAs[j] = ps2.tile([P, F], dt, tag="pA", name="pA")
                    pBs[j] = ps2.tile([P, F], dt, tag="pB", name="pB")
                    nc.tensor.matmul(pAs[j][:, :], tEa[32 * j:32 * (j + 1), :],
                                     saT[32 * j:32 * (j + 1), :],
                                     start=True, stop=True,
                                     tile_position=(32 * j, 0))
                    nc.tensor.matmul(pBs[j][:, :], tEb[32 * j:32 * (j + 1), :],
                                     sbT[32 * j:32 * (j + 1), :],
                                     start=True, stop=True,
                                     tile_position=(32 * j, 0))

                emit_exp(0)
                emit_exp(1)
                spps = [None] * 4
                for j in range(4):
                    sA = sb.tile([P, F], dm, tag="sA", name="sA")
                    nc.scalar.copy(sA[:, :], pAs[j][:, :])
                    spps[j] = sb.tile([P, F], dm, tag="spp", name="spp", bufs=5)
                    nc.vector.tensor_mul(spps[j][:, :], sA[:, :], pBs[j][:, :])
                    if j + 2 < 4:
                        emit_exp(j + 2)
                pout2 = ps1.tile([P, F], dt, tag="pout2", name="pout2")
                for j in range(4):
                    nc.tensor.matmul(pout2[32 * j:32 * (j + 1), :], tK4[:, :],
                                     spps[j][:, :], start=True, stop=True,
                                     tile_position=(0, 32 * j))
                return pout2

            def emit_back(n, pout2):
                do = dm if obf else dt
                sout2 = sb.tile([P, F], do, tag="sout2", name="sout2")
                nc.scalar.copy(sout2[:, :], pout2[:, :])
                if ablate >= 2:
                    nc.sync.dma_start(of[n], sout2[:, :])
                    return
                poTs = ps1.tile([P, F], do, tag="pout2", name="poTs")
                for c in range(4):
                    nc.tensor.transpose(poTs[:, 128 * c:128 * (c + 1)],
                                        sout2[:, 128 * c:128 * (c + 1)],
                                        tI[:, :] if obf else tI32[:, :])
                onat = sb.tile([P, F], dt, tag="onat", name="onat")
                nc.vector.tensor_copy(onat[:, :], poTs[:, :])
                nc.sync.dma_start(of[n], onat[:, :])

            loop_cm = (tc.For_i(0, dyn_repeats, 1) if dyn_repeats
                       else contextlib.nullcontext())
            with loop_cm:
              for _rep in range(repeats):
                prev = None
                for n in range(nt):
                    if back_first and prev is not None:
                        emit_back(prev[0], prev[1])
                        prev = None
                    pout2 = emit_front(n)
                    if prev is not None:
                        emit_back(prev[0], prev[1])
                    prev = (n, pout2) if pout2 is not None else None
                if prev is not None:
                    emit_back(prev[0], prev[1])

    nc.finalize()
    return nc


def build_program2(rows_per_core=ROWS_PER_CORE, dyn_repeats=None, mega_q=8,
                   ablate=0, obf=True, store_ring="gpsimd", sb_bufs=4,
                   cast_engine="pool"):
    """v2: megatile DMA (mega_q big-tiles per DMA) spread over SP/ACT/SWDGE
    rings; f32 loads; f32->bf16 cast on Pool; bf16 compute; bf16 out
    transpose (obf).

    Per-DMA fixed cost is ~2us (completion latency), so 256KB-grain DMAs
    ran at ~98GB/s effective. mega_q=8 -> 2MB grains, 12 DMAs/pass."""
    import contextlib

    import concourse.bacc as bacc
    import concourse.mybir as mybir
    from concourse.tile import TileContext

    nt = rows_per_core // 4096
    nm = nt // mega_q
    assert nm * mega_q == nt
    FM = 512 * mega_q                     # f32 elems per partition per mega
    nc = bacc.Bacc("TRN2", target_bir_lowering=False)
    dt = mybir.dt.float32
    dm = mybir.dt.bfloat16
    a = nc.dram_tensor("a", [rows_per_core, 16], dt, kind="ExternalInput")
    b = nc.dram_tensor("b", [rows_per_core, 16], dt, kind="ExternalInput")
    cEa = nc.dram_tensor("cEa", [128, 128], dm, kind="ExternalInput")
    cEb = nc.dram_tensor("cEb", [128, 128], dm, kind="ExternalInput")
    cK4 = nc.dram_tensor("cK4", [128, 32], dm, kind="ExternalInput")
    cI = nc.dram_tensor("cI", [128, 128], dm, kind="ExternalInput")
    cI32 = nc.dram_tensor("cI32", [128, 128], dt, kind="ExternalInput")
    o = nc.dram_tensor("o", [rows_per_core, 16], dt, kind="ExternalOutput")

    af = a.rearrange("(m g W) c -> m g (W c)", g=P, W=32 * mega_q)
    bf = b.rearrange("(m g W) c -> m g (W c)", g=P, W=32 * mega_q)
    of = o.rearrange("(m g W) c -> m g (W c)", g=P, W=32 * mega_q)

    with TileContext(nc) as tc:
        with tc.tile_pool(name="const", bufs=1) as cpool, \
             tc.tile_pool(name="mega", bufs=2) as mg, \
             tc.tile_pool(name="sb", bufs=sb_bufs) as sb, \
             tc.tile_pool(name="ps1", bufs=2, space="PSUM") as ps1, \
             tc.tile_pool(name="ps2", bufs=2, space="PSUM") as ps2:
            tEa = cpool.tile([128, 128], dm)
            tEb = cpool.tile([128, 128], dm)
            tK4 = cpool.tile([128, 32], dm)
            tI = cpool.tile([128, 128], dm)
            tI32 = cpool.tile([128, 128], dt)
            nc.sync.dma_start(tEa[:, :], cEa[:, :])
            nc.sync.dma_start(tEb[:, :], cEb[:, :])
            nc.sync.dma_start(tK4[:, :], cK4[:, :])
            nc.sync.dma_start(tI[:, :], cI[:, :])
            nc.sync.dma_start(tI32[:, :], cI32[:, :])
            store_q = {"gpsimd": nc.gpsimd, "sync": nc.sync,
                       "scalar": nc.scalar, "vector": nc.vector}[store_ring]
            cast_q = {"pool": nc.gpsimd, "vector": nc.vector,
                      "scalar": nc.scalar, "none": None}[cast_engine]

            def emit_mega(m):
                ma = mg.tile([P, FM], dt, tag="ma", name="ma")
                mb = mg.tile([P, FM], dt, tag="mb", name="mb")
                mo = mg.tile([P, FM], dt, tag="mo", name="mo")
                nc.sync.dma_start(ma[:, :], af[m])
                nc.scalar.dma_start(mb[:, :], bf[m])
                if ablate >= 6:
                    nc.vector.tensor_copy(mo[:, 0:FM], ma[:, 0:FM])
                    store_q.dma_start(of[m], mo[:, :])
                    return

                for q in range(mega_q):
                    qs = slice(512 * q, 512 * (q + 1))
                    if cast_q is not None:
                        # cast f32 -> bf16 on an otherwise idle engine
                        ta = sb.tile([P, F], dm, tag="ta", name="ta")
                        tb = sb.tile([P, F], dm, tag="tb", name="tb")
                        cast_q.tensor_copy(ta[:, :], ma[:, qs])
                        cast_q.tensor_copy(tb[:, :], mb[:, qs])
                        ta_v, tb_v, tT, dT = ta[:, :], tb[:, :], tI, dm
                    else:
                        # f32 transposes (2 cyc/row on PE); cast to bf16 in
                        # the PSUM->SBUF evacuation copy
                        ta_v, tb_v, tT, dT = ma[:, qs], mb[:, qs], tI32, dt

                    saT = sb.tile([P, F], dm, tag="saT", name="saT")
                    sbT = sb.tile([P, F], dm, tag="sbT", name="sbT")
                    paT = ps1.tile([P, F], dT, tag="pT", name="paT")
                    for c in range(4):
                        nc.tensor.transpose(paT[:, 128 * c:128 * (c + 1)],
                                            ta_v[:, 128 * c:128 * (c + 1)], tT[:, :])
                    nc.scalar.copy(saT[:, :], paT[:, :])
                    pbT = ps1.tile([P, F], dT, tag="pT", name="pbT")
                    for c in range(4):
                        nc.tensor.transpose(pbT[:, 128 * c:128 * (c + 1)],
                                            tb_v[:, 128 * c:128 * (c + 1)], tT[:, :])
                    nc.vector.tensor_copy(sbT[:, :], pbT[:, :])
                    if ablate >= 5:
                        nc.vector.tensor_copy(mo[:, qs], saT[:, :])
                        if q == mega_q - 1:
                            store_q.dma_start(of[m], mo[:, :])
                        continue

                    pAs, pBs = [None] * 4, [None] * 4

                    def emit_exp(j):
                        pAs[j] = ps2.tile([P, F], dt, tag="pA", name="pA")
                        pBs[j] = ps2.tile([P, F], dt, tag="pB", name="pB")
                        nc.tensor.matmul(pAs[j][:, :], tEa[32 * j:32 * (j + 1), :],
                                         saT[32 * j:32 * (j + 1), :],
                                         start=True, stop=True,
                                         tile_position=(32 * j, 0))
                        nc.tensor.matmul(pBs[j][:, :], tEb[32 * j:32 * (j + 1), :],
                                         sbT[32 * j:32 * (j + 1), :],
                                         start=True, stop=True,
                                         tile_position=(32 * j, 0))

                    emit_exp(0)
                    emit_exp(1)
                    spps = [None] * 4
                    for j in range(4):
                        sA = sb.tile([P, F], dm, tag="sA", name="sA")
                        nc.scalar.copy(sA[:, :], pAs[j][:, :])
                        spps[j] = sb.tile([P, F], dm, tag="spp", name="spp", bufs=5)
                        nc.vector.tensor_mul(spps[j][:, :], sA[:, :], pBs[j][:, :])
                        if j + 2 < 4:
                            emit_exp(j + 2)
                    pout2 = ps1.tile([P, F], dt, tag="pout2", name="pout2")
                    for j in range(4):
                        nc.tensor.matmul(pout2[32 * j:32 * (j + 1), :], tK4[:, :],
                                         spps[j][:, :], start=True, stop=True,
                                         tile_position=(0, 32 * j))

                    do = dm if obf else dt
                    sout2 = sb.tile([P, F], do, tag="sout2", name="sout2")
                    nc.scalar.copy(sout2[:, :], pout2[:, :])
                    if ablate >= 2:
                        nc.vector.tensor_copy(mo[:, qs], sout2[:, :])
                        if q == mega_q - 1:
                            store_q.dma_start(of[m], mo[:, :])
                        continue
                    poTs = ps1.tile([P, F], do, tag="pout2", name="poTs")
                    for c in range(4):
                        nc.tensor.transpose(poTs[:, 128 * c:128 * (c + 1)],
                                            sout2[:, 128 * c:128 * (c + 1)],
                                            tI[:, :] if obf else tI32[:, :])
                    nc.vector.tensor_copy(mo[:, qs], poTs[:, :])
                    if q == mega_q - 1:
                        store_q.dma_start(of[m], mo[:, :])

            loop_cm = (tc.For_i(0, dyn_repeats, 1) if dyn_repeats
                       else contextlib.nullcontext())
            with loop_cm:
                for m in range(nm):
                    emit_mega(m)

    nc.finalize()
    return nc


def build_program3(rows_per_core=ROWS_PER_CORE, dyn_repeats=None, mega_q=8,
                   ablate=0, store_ring="gpsimd", sb_bufs=3,
                   cast_a="gpsimd", cast_b="gpsimd", mo_eng="vector",
                   sa_split=0, mul_split=0):
    """v3: 1024-wide (subtile-pair) elementwise ops, all-bf16 PSUM, megatile
    DMA on SP/ACT rings with stores on SWDGE. Engine assignment tunable:
    cast_a/cast_b: engine for the f32->bf16 pre-casts ('none' folds the cast
    into the transpose at 2cyc/row). sa_split: how many of the 4 sA
    evacuations go to gpsimd instead of scalar. mul_split: how many of the 4
    muls go to gpsimd instead of vector."""
    import contextlib

    import concourse.bacc as bacc
    import concourse.mybir as mybir
    from concourse.tile import TileContext

    nt = rows_per_core // 4096
    nm = nt // mega_q
    assert nm * mega_q == nt and mega_q % 2 == 0
    FM = 512 * mega_q
    F2 = 1024
    nc = bacc.Bacc("TRN2", target_bir_lowering=False)
    dt = mybir.dt.float32
    dm = mybir.dt.bfloat16
    a = nc.dram_tensor("a", [rows_per_core, 16], dt, kind="ExternalInput")
    b = nc.dram_tensor("b", [rows_per_core, 16], dt, kind="ExternalInput")
    cEa = nc.dram_tensor("cEa", [128, 128], dm, kind="ExternalInput")
    cEb = nc.dram_tensor("cEb", [128, 128], dm, kind="ExternalInput")
    cK4 = nc.dram_tensor("cK4", [128, 32], dm, kind="ExternalInput")
    cI = nc.dram_tensor("cI", [128, 128], dm, kind="ExternalInput")
    cI32 = nc.dram_tensor("cI32", [128, 128], dt, kind="ExternalInput")
    o = nc.dram_tensor("o", [rows_per_core, 16], dt, kind="ExternalOutput")

    af = a.rearrange("(m g W) c -> m g (W c)", g=P, W=32 * mega_q)
    bf = b.rearrange("(m g W) c -> m g (W c)", g=P, W=32 * mega_q)
    of = o.rearrange("(m g W) c -> m g (W c)", g=P, W=32 * mega_q)

    def eng(name):
        return {"scalar": nc.scalar, "vector": nc.vector,
                "gpsimd": nc.gpsimd, "none": None}[name]

    def copy_on(q, dst, src):
        if q is nc.scalar:
            q.copy(dst, src)
        else:
            q.tensor_copy(dst, src)

    with TileContext(nc) as tc:
        with tc.tile_pool(name="const", bufs=1) as cpool, \
             tc.tile_pool(name="mega", bufs=2) as mg, \
             tc.tile_pool(name="sb", bufs=sb_bufs) as sb, \
             tc.tile_pool(name="ps1", bufs=2, space="PSUM") as ps1, \
             tc.tile_pool(name="ps2", bufs=2, space="PSUM") as ps2:
            tEa = cpool.tile([128, 128], dm)
            tEb = cpool.tile([128, 128], dm)
            tK4 = cpool.tile([128, 32], dm)
            tI = cpool.tile([128, 128], dm)
            tI32 = cpool.tile([128, 128], dt)
            nc.sync.dma_start(tEa[:, :], cEa[:, :])
            nc.sync.dma_start(tEb[:, :], cEb[:, :])
            nc.sync.dma_start(tK4[:, :], cK4[:, :])
            nc.sync.dma_start(tI[:, :], cI[:, :])
            nc.sync.dma_start(tI32[:, :], cI32[:, :])
            store_q = {"gpsimd": nc.gpsimd, "sync": nc.sync,
                       "scalar": nc.scalar}[store_ring]
            ca_q, cb_q, mo_q = eng(cast_a), eng(cast_b), eng(mo_eng)

            def emit_pair(m, mo, qp):
                qs = slice(F2 * qp, F2 * (qp + 1))
                ma, mb = emit_pair.ma, emit_pair.mb

                def tsrc(which, cq):
                    msrc = ma if which == "a" else mb
                    if cq is None:
                        return msrc[:, qs], tI32, dt
                    t = sb.tile([P, F2], dm, tag="t" + which, name="t" + which)
                    copy_on(cq, t[:, :], msrc[:, qs])
                    return t[:, :], tI, dm

                ta_v, tTa, dTa = tsrc("a", ca_q)
                tb_v, tTb, dTb = tsrc("b", cb_q)

                saT = sb.tile([P, F2], dm, tag="saT", name="saT")
                sbT = sb.tile([P, F2], dm, tag="sbT", name="sbT")
                paT = ps1.tile([P, F2], dm, tag="pT", name="paT")
                for c in range(8):
                    nc.tensor.transpose(paT[:, 128 * c:128 * (c + 1)],
                                        ta_v[:, 128 * c:128 * (c + 1)], tTa[:, :])
                nc.scalar.copy(saT[:, :], paT[:, :])
                pbT = ps1.tile([P, F2], dm, tag="pT", name="pbT")
                for c in range(8):
                    nc.tensor.transpose(pbT[:, 128 * c:128 * (c + 1)],
                                        tb_v[:, 128 * c:128 * (c + 1)], tTb[:, :])
                nc.vector.tensor_copy(sbT[:, :], pbT[:, :])
                if ablate >= 5:
                    nc.vector.tensor_copy(mo[:, qs], saT[:, :])
                    return

                # expansion / mul stage runs 512-wide per half (PSUM budget:
                # fp32 matmul outputs), K4 lands in one 1024-wide fp32 psum
                pAs, pBs = {}, {}

                def emit_exp(h, j):
                    hs = slice(F * h, F * (h + 1))
                    pAs[h, j] = ps2.tile([P, F], dt, tag="pA", name="pA")
                    pBs[h, j] = ps2.tile([P, F], dt, tag="pB", name="pB")
                    nc.tensor.matmul(pAs[h, j][:, :], tEa[32 * j:32 * (j + 1), :],
                                     saT[32 * j:32 * (j + 1), hs],
                                     start=True, stop=True,
                                     tile_position=(32 * j, 0))
                    nc.tensor.matmul(pBs[h, j][:, :], tEb[32 * j:32 * (j + 1), :],
                                     sbT[32 * j:32 * (j + 1), hs],
                                     start=True, stop=True,
                                     tile_position=(32 * j, 0))

                hj = [(h, j) for h in range(2) for j in range(4)]
                emit_exp(*hj[0])
                emit_exp(*hj[1])
                spps = {}
                pout2 = ps1.tile([P, F2], dt, tag="pout2", name="pout2", bufs=1)
                for i, (h, j) in enumerate(hj):
                    sa_q = nc.gpsimd if j < sa_split else nc.scalar
                    mu_q = nc.gpsimd if j < mul_split else nc.vector
                    sA = sb.tile([P, F], dm, tag="sA", name="sA", bufs=5)
                    copy_on(sa_q, sA[:, :], pAs[h, j][:, :])
                    spps[h, j] = sb.tile([P, F], dm, tag="spp", name="spp",
                                         bufs=6)
                    mu_q.tensor_mul(spps[h, j][:, :], sA[:, :], pBs[h, j][:, :])
                    if i + 2 < 8:
                        emit_exp(*hj[i + 2])
                for h in range(2):
                    for j in range(4):
                        nc.tensor.matmul(
                            pout2[32 * j:32 * (j + 1), F * h:F * (h + 1)],
                            tK4[:, :], spps[h, j][:, :], start=True, stop=True,
                            tile_position=(0, 32 * j))

                sout2 = sb.tile([P, F2], dm, tag="sout2", name="sout2")
                nc.scalar.copy(sout2[:, :], pout2[:, :])
                if ablate >= 2:
                    nc.vector.tensor_copy(mo[:, qs], sout2[:, :])
                    return
                poTs = ps1.tile([P, F2], dm, tag="pT", name="poTs")
                for c in range(8):
                    nc.tensor.transpose(poTs[:, 128 * c:128 * (c + 1)],
                                        sout2[:, 128 * c:128 * (c + 1)], tI[:, :])
                copy_on(mo_q, mo[:, qs], poTs[:, :])

            def emit_mega(m):
                ma = mg.tile([P, FM], dt, tag="ma", name="ma")
                mb = mg.tile([P, FM], dt, tag="mb", name="mb")
                mo = mg.tile([P, FM], dt, tag="mo", name="mo")
                nc.sync.dma_start(ma[:, :], af[m])
                nc.scalar.dma_start(mb[:, :], bf[m])
                if ablate >= 6:
                    nc.vector.tensor_copy(mo[:, 0:FM], ma[:, 0:FM])
                else:
                    emit_pair.ma, emit_pair.mb = ma, mb
                    for qp in range(mega_q // 2):
                        emit_pair(m, mo, qp)
                store_q.dma_start(of[m], mo[:, :])

            loop_cm = (tc.For_i(0, dyn_repeats, 1) if dyn_repeats
                       else contextlib.nullcontext())
            with loop_cm:
                for m in range(nm):
                    emit_mega(m)

    nc.finalize()
    return nc


_CACHE = {}


def make_in_maps(a, b, bf16=False):
    import ml_dtypes
    Ea4, Eb4, K4c, I128 = _build_consts()
    md = ml_dtypes.bfloat16 if bf16 else np.float32
    consts = {"cEa": Ea4.astype(md), "cEb": Eb4.astype(md),
              "cK4": K4c.astype(md), "cI": I128.astype(md), "cI32": I128}
    in_maps = []
    for i in range(N_CORES):
        sl = slice(i * ROWS_PER_CORE, (i + 1) * ROWS_PER_CORE)
        in_maps.append({"a": a[sl], "b": b[sl], **consts})
    return in_maps


USE_BF16 = False

# Final program config ("v2" or "v3" + builder kwargs). build_final is also
# used by test.py so the timed program is exactly the one kernel() runs.
FINAL = ("v2", dict(cast_engine="none", store_ring="sync"))


def build_final(dyn_repeats=None):
    ver, kw = FINAL
    builder = {"v2": build_program2, "v3": build_program3}[ver]
    return builder(dyn_repeats=dyn_repeats, **kw)


def kernel(a, b):
    from concourse.bass_utils import run_bass_kernel_spmd

    a = np.ascontiguousarray(np.asarray(a, dtype=np.float32))
    b = np.ascontiguousarray(np.asarray(b, dtype=np.float32))
    assert a.shape == (N_TOTAL, 16) and b.shape == (N_TOTAL, 16)
    if "nc" not in _CACHE:
        _CACHE["nc"] = build_final()
    nc = _CACHE["nc"]
    in_maps = make_in_maps(a, b, bf16=True)
    res = run_bass_kernel_spmd(nc, in_maps, core_ids=list(range(N_CORES)))
    return np.concatenate([res.results[i]["o"] for i in range(N_CORES)], axis=0)

